# revision 3
# baseline (speedup 1.0000x reference)
"""GCN (3-layer) + global mean pool on 8 Trainium2 NeuronCores.

Sharding: 1024 graphs -> 8 shards of 128 graphs (contiguous node ranges,
batch is sorted). Each core owns its shard's nodes and all edges whose dst
lands in the shard. Per layer, each core computes the per-node linear table
T = dinv * (h @ W) for its own nodes, an AllGather replicates the full table,
then dma_gather pulls T[src] rows per edge (ELL slots per 128-node tile,
split by int16 chunk of the global table) and a strided DVE reduce sums each
node's slots. norm factorizes as dinv[src]*dinv[dst]: dinv[src] is folded
into the table, dinv[dst] is applied after the reduce.
"""

import numpy as np

N_NODES = 100000
N_GRAPHS = 1024
N_CORES = 8
GPC = N_GRAPHS // N_CORES  # graphs per core
F = 32                     # true feature width
PF = 64                    # padded row width (256B dma_gather elem)
CHUNK = 32768              # int16-addressable table rows per gather chunk
BN_EPS = 1e-5
import os
MAX_SLOTS_PER_CALL = int(os.environ.get("K_SLOTS", "8"))  # 1024-idx dma_gather runtime cap / 128
N_QUEUES = int(os.environ.get("K_QUEUES", "4"))
SCRATCH = int(os.environ.get("K_SCRATCH", "65536"))

_CACHE = {}


# --------------------------------------------------------------------------
# host-side prep: sharding, ELL layout, wrapped index image
# --------------------------------------------------------------------------

def _prep(edge_index, batch):
    src_g = edge_index[0].astype(np.int64)
    dst_g = edge_index[1].astype(np.int64)
    batch = batch.astype(np.int64)
    n = N_NODES

    # in-degree including self loop (= reference deg)
    deg = np.bincount(dst_g, minlength=n).astype(np.int64) + 1

    # node ranges per core: graphs [128c, 128c+128)
    gcounts = np.bincount(batch, minlength=N_GRAPHS)
    gends = np.cumsum(gcounts)
    st = np.zeros(N_CORES + 1, np.int64)
    for c in range(1, N_CORES + 1):
        st[c] = gends[GPC * c - 1]
    sizes = np.diff(st)
    S = (int(sizes.max()) // 128 + 1) * 128  # >=1 pad row per shard
    T = S // 128

    # per-core degree sort (desc) -> local position of each node
    loc_of = np.empty(n, np.int64)
    for c in range(N_CORES):
        j = np.arange(st[c], st[c + 1])
        order = np.argsort(-deg[j], kind="stable")
        loc_of[j[order]] = np.arange(sizes[c])
    owner = np.searchsorted(st[1:], np.arange(n), side="right")
    owner_of = lambda v: np.searchsorted(st[1:], v, side="right")
    row_of = S * owner + loc_of  # global table row of node

    # all edges incl self loops, routed to dst owner
    all_src = np.concatenate([src_g, np.arange(n)])
    all_dst = np.concatenate([dst_g, np.arange(n)])
    e_owner = owner[all_dst]
    e_srcrow = row_of[all_src]
    e_dstloc = loc_of[all_dst]

    # pad (zero) rows per chunk: any never-written-by-a-real-node row < 8S
    used = np.zeros(N_CORES * S, bool)
    used[row_of] = True
    pad_rows = np.nonzero(~used)[0]
    Zq = np.empty(4, np.int64)
    for q in range(4):
        cand = pad_rows[(pad_rows >= CHUNK * q) & (pad_rows < CHUNK * (q + 1))]
        assert len(cand) > 0, f"no pad row in chunk {q}"
        Zq[q] = cand[0]

    # per-core (tile, chunk) slot counts -> common maxima
    counts = np.zeros((N_CORES, T * 128, 4), np.int32)
    core_edges = []
    for c in range(N_CORES):
        m = e_owner == c
        sr, dl = e_srcrow[m], e_dstloc[m]
        q = sr // CHUNK
        np.add.at(counts[c], (dl, q), 1)
        core_edges.append((sr, dl, q))
    cnt_tiles = counts.reshape(N_CORES, T, 128, 4)
    stq = cnt_tiles.max(axis=(0, 2)).astype(np.int64)  # [T, 4] common
    D = stq.sum(axis=1)                                # [T] slots per tile
    qoff = np.cumsum(stq, axis=1) - stq                # [T, 4] slot offset of chunk q
    tile_off = np.concatenate([[0], np.cumsum(D)])     # [T+1]
    TOT = int(tile_off[-1])

    # gather call structure (common): per tile, list of (q, slot_off, k_slots)
    calls = []
    for t in range(T):
        cl = []
        for q in range(4):
            s = int(stq[t, q])
            so = int(qoff[t, q])
            while s > 0:
                k = min(MAX_SLOTS_PER_CALL, s)
                cl.append((q, so, k))
                so += k
                s -= k
        calls.append(cl)

    # per-core flat ELL image [128, TOT] int16 (slot-major per tile)
    imgs = []
    for c in range(N_CORES):
        sr, dl, q = core_edges[c]
        img = np.empty((128, TOT), np.int16)
        for t in range(T):
            for qq in range(4):
                img[:, tile_off[t] + qoff[t, qq] : tile_off[t] + qoff[t, qq] + stq[t, qq]] = (
                    Zq[qq] - CHUNK * qq
                )
        # rank of edge within its (dst, chunk) group
        key = dl * 4 + q
        order = np.argsort(key, kind="stable")
        sk = key[order]
        starts = np.concatenate([[0], np.nonzero(np.diff(sk))[0] + 1])
        grp_start = np.zeros(len(sk), np.int64)
        grp_start[starts] = np.arange(len(sk))[starts]
        grp_start = np.maximum.accumulate(grp_start)
        rank = np.arange(len(sk)) - grp_start
        sro, dlo, qo = sr[order], dl[order], q[order]
        t_ = dlo // 128
        p_ = dlo % 128
        slot = qoff[t_, qo] + rank
        img[p_, tile_off[t_] + slot] = (sro - CHUNK * qo).astype(np.int16)
        imgs.append(img)

    # wrap for dma_gather idx layout: per call, [128, 8k] block where
    # [pp, i] = logical[(i*16 + pp%16)]  with logical j -> (slot j//128, part j%128)
    wrap_cols = 8 * TOT
    wimgs = [np.empty((128, wrap_cols), np.int16) for _ in range(N_CORES)]
    pp = np.arange(128)[:, None]
    woff_of_call = []  # per tile: list of wrapped col offsets aligned with calls
    for t in range(T):
        woffs = []
        for (q, so, k) in calls[t]:
            woffs.append(8 * (tile_off[t] + so))
            i = np.arange(8 * k)[None, :]
            j = i * 16 + (pp % 16)
            for c in range(N_CORES):
                blk = imgs[c][:, tile_off[t] + so : tile_off[t] + so + k]
                wimgs[c][:, 8 * (tile_off[t] + so) : 8 * (tile_off[t] + so + k)] = blk[
                    j % 128, j // 128
                ]
        woff_of_call.append(woffs)

    # per-core aux arrays (local order, padded to S)
    deg_loc = np.zeros((N_CORES, S, 1), np.float32)
    bat_loc = np.full((N_CORES, S, 1), 1000.0, np.float32)
    cnt_loc = np.zeros((N_CORES, 128, 1), np.float32)
    for c in range(N_CORES):
        j = np.arange(st[c], st[c + 1])
        deg_loc[c, loc_of[j], 0] = deg[j]
        bat_loc[c, loc_of[j], 0] = batch[j] - GPC * c
        cnt_loc[c, :, 0] = gcounts[GPC * c : GPC * (c + 1)]

    return dict(
        S=S, T=T, stq=stq, D=D, tile_off=tile_off, TOT=TOT, calls=calls,
        woff_of_call=woff_of_call, wimgs=wimgs, imgs=imgs, qoff=qoff,
        deg_loc=deg_loc, bat_loc=bat_loc, cnt_loc=cnt_loc, loc_of=loc_of,
        st=st, Zq=Zq, row_of=row_of,
    )


# --------------------------------------------------------------------------
# walrus workaround: at most one sem-wait per instruction
# --------------------------------------------------------------------------

def _install_tile_patch():
    import concourse.mybir as mybir
    from concourse.tile import TileContext
    from concourse.vector_clock import ScopedClock

    if getattr(TileContext, "_wait_split_installed", False):
        return

    def split_all_waits(nc):
        for bb in nc.main_func.blocks:
            insts = list(bb.instructions)
            if not any(
                i.sync_info is not None and len(i.sync_info.on_wait) > 1
                for i in insts
            ):
                continue
            newlist = []
            tail_bb = nc.cur_bb.bb if nc.cur_bb is not None else None
            for inst in insts:
                w = list(inst.sync_info.on_wait) if inst.sync_info is not None else []
                if len(w) > 1 and inst.engine != mybir.EngineType.Unassigned:
                    extra, keep = w[:-1], w[-1:]
                    inst.sync_info.on_wait = keep
                    eng = nc.engines[inst.engine]
                    for wi in extra:
                        nop = eng.nop(nofuse=True, hint="wait_split")
                        ni = nop.ins if hasattr(nop, "ins") else nop
                        if tail_bb is not None and ni in tail_bb.instructions:
                            tail_bb.instructions.remove(ni)
                        if ni.sync_info is None:
                            ni.sync_info = mybir.SyncInfo(on_wait=[], on_update=[])
                        ni.sync_info.on_wait = [wi]
                        ni.sync_info.on_update = []
                        newlist.append(ni)
                newlist.append(inst)
            bb.instructions.clear()
            for x in newlist:
                bb.instructions.append(x)

    def _patched(self, tick_clock, wait_clock):
        drain_inst = self.nc.sync.drain()
        wait_clock.add_sem_waits(
            drain_inst.ins, ScopedClock({None: tick_clock.global_clock})
        )
        self.nc.all_engine_barrier()
        assert self.sems is not None
        popped = self.nc._tile_sem_poison_stack.pop()
        assert popped is self._sem_poison
        self.nc.clear_and_free_semaphores(list(self.sems.allocated().values()))
        self.nc.all_engine_barrier()
        _requeue_gather_sems(self.nc)
        split_all_waits(self.nc)

    TileContext._drain_and_barrier = _patched
    TileContext._wait_split_installed = True


def _requeue_gather_sems(nc):
    """Make multi-queue dma_gather sound.

    The Tile scheduler counts all SWDGE completions on rotating DMASW sems and
    waits assume FIFO completion order - false across queues. Rewrite: each
    gather updates a per-queue cumulative sem; every consumer wait on a DMASW
    sem (value 16m = "m-th update of that sem") is mapped to its program-order
    dependency prefix N and replaced by per-queue waits (count of prefix on
    each queue). Producer-side sem-slot-reuse waits are dropped (cumulative
    sems are never reset mid-program).
    """
    import copy
    if N_QUEUES <= 1 or not os.environ.get("K_SURGERY"):
        return
    gathers = []
    for bb in nc.main_func.blocks:
        for inst in bb.instructions:
            if type(inst).__name__ == "InstDMAGatherAnt":
                gathers.append(inst)
    if not gathers:
        return
    swsem_ids = set()
    for g in gathers:
        for u in g.sync_info.on_update:
            if str(getattr(u, "ant_name", "")).startswith("DMASW"):
                swsem_ids.add(u.id)
    # no non-gather instruction may update these sems mid-program
    for bb in nc.main_func.blocks:
        for inst in bb.instructions:
            if type(inst).__name__ == "InstDMAGatherAnt" or inst.sync_info is None:
                continue
            for u in inst.sync_info.on_update:
                assert getattr(u, "id", None) not in swsem_ids, (
                    f"non-gather updates DMASW sem: {inst}"
                )
    upd_order = {s: [] for s in swsem_ids}
    for i, g in enumerate(gathers):
        for u in g.sync_info.on_update:
            if u.id in swsem_ids:
                upd_order[u.id].append(i)
    qsems = sorted(swsem_ids)[:N_QUEUES]
    q_of = [int(g.queue_num) for g in gathers]
    prefix = [(0,) * N_QUEUES]
    cur = [0] * N_QUEUES
    for q in q_of:
        cur[q] += 1
        prefix.append(tuple(cur))

    proto_wait = None
    for bb in nc.main_func.blocks:
        for inst in bb.instructions:
            if inst.sync_info is None:
                continue
            for w in inst.sync_info.on_wait:
                if getattr(w, "id", None) in swsem_ids:
                    proto_wait = w
                    break
            if proto_wait is not None:
                break
        if proto_wait is not None:
            break

    # rewrite gather updates to per-queue cumulative sems; drop their DMASW waits
    # (replace with fresh copies - sync objects may be shared across insts)
    for i, g in enumerate(gathers):
        new_upd = []
        for u in g.sync_info.on_update:
            if u.id in swsem_ids:
                nu = copy.deepcopy(u)
                nu.id = qsems[q_of[i]]
                nu.ant_name = f"QSW{q_of[i]}"
                new_upd.append(nu)
            else:
                new_upd.append(u)
        g.sync_info.on_update = new_upd
        g.sync_info.on_wait = [
            w for w in g.sync_info.on_wait if getattr(w, "id", None) not in swsem_ids
        ]

    # rewrite consumer waits
    for bb in nc.main_func.blocks:
        for inst in bb.instructions:
            if type(inst).__name__ == "InstDMAGatherAnt" or inst.sync_info is None:
                continue
            waits = list(inst.sync_info.on_wait)
            sw = [w for w in waits if getattr(w, "id", None) in swsem_ids]
            if not sw:
                continue
            N = 0
            for w in sw:
                m = int(w.wait_value) // 16
                order = upd_order[w.id]
                if m <= 0 or not order:
                    continue
                m = min(m, len(order))
                N = max(N, order[m - 1] + 1)
            keep = [w for w in waits if getattr(w, "id", None) not in swsem_ids]
            for q in range(N_QUEUES):
                c = prefix[N][q]
                if c > 0:
                    nw = copy.deepcopy(proto_wait)
                    nw.id = qsems[q]
                    nw.ant_name = f"QSW{q}"
                    nw.wait_value = 16 * c
                    keep.append(nw)
            inst.sync_info.on_wait = keep


# --------------------------------------------------------------------------
# device program
# --------------------------------------------------------------------------

def _build(meta, n_layers=3, do_gather=True, do_reduce=True, do_tables=True, do_ag=True):
    import concourse.bacc as bacc
    import concourse.mybir as mybir
    from concourse.tile import TileContext

    _install_tile_patch()

    S, T = meta["S"], meta["T"]
    calls, woffs, tile_off, D = (
        meta["calls"], meta["woff_of_call"], meta["tile_off"], meta["D"],
    )
    TOT = meta["TOT"]
    NT = N_CORES * S  # real table rows
    f32 = mybir.dt.float32

    nc = bacc.Bacc(None, target_bir_lowering=False, num_swdge_queues=N_QUEUES,
                   dynamic_dma_scratch_size=SCRATCH)
    P_ = nc.declare_dram_parameter

    x_row = P_("x_row", [1, S], f32, isOutput=False)
    deg_p = P_("deg", [128, S // 128], f32, isOutput=False)
    bat_p = P_("bat", [128, S // 128], f32, isOutput=False)
    cnt_p = P_("cnt", [128, 1], f32, isOutput=False)
    idx_p = P_("idximg", [128, 8 * TOT], mybir.dt.int16, isOutput=False)
    w1_p = P_("W1p", [1, PF], f32, isOutput=False)
    w2_p = P_("W2p", [F, PF], f32, isOutput=False)
    w3_p = P_("W3p", [F, PF], f32, isOutput=False)
    bnsc_in = [None, P_("bnsc1", [PF, 1], f32, isOutput=False),
               P_("bnsc2", [PF, 1], f32, isOutput=False)]
    bnsh_in = [None, P_("bnsh1", [PF, 1], f32, isOutput=False),
               P_("bnsh2", [PF, 1], f32, isOutput=False)]
    b3_p = P_("b3", [PF, 1], f32, isOutput=False)
    iota_p = P_("iota", [1, 128], f32, isOutput=False)
    ones_p = P_("ones", [1, 128], f32, isOutput=False)
    out_p = P_("out", [F, 128], f32, isOutput=True)

    tloc = nc.dram_tensor("tloc", [S, PF], f32)
    # double-buffered gather table: AG for layer L+1 must not overwrite the
    # buffer still being read by layer L's (queue-reordered) gathers
    tabs = [
        nc.dram_tensor("tabA", [4 * CHUNK, PF], f32, addr_space="Shared"),
        nc.dram_tensor("tabB", [4 * CHUNK, PF], f32, addr_space="Shared"),
    ]

    with TileContext(nc) as tc:
        with (
            tc.tile_pool(name="const", bufs=1) as cpool,
            tc.tile_pool(name="work", bufs=3) as wpool,
            tc.tile_pool(name="msg", bufs=3) as mpool,
            tc.tile_pool(name="psum", bufs=2, space="PSUM") as ppool,
            tc.tile_pool(name="psum1", bufs=1, space="PSUM") as ppool1,
        ):
            # ---- constants ----
            w1 = cpool.tile([1, PF], f32, tag="w1")
            nc.sync.dma_start(out=w1[:], in_=w1_p[:])
            w2 = cpool.tile([F, PF], f32, tag="w2")
            nc.sync.dma_start(out=w2[:], in_=w2_p[:])
            w3 = cpool.tile([F, PF], f32, tag="w3")
            nc.sync.dma_start(out=w3[:], in_=w3_p[:])
            bnsc = [None, None, None]
            bnsh = [None, None, None]
            for L in (1, 2):
                bnsc[L] = cpool.tile([PF, 1], f32, tag=f"bnsc{L}", name=f"bnsc{L}")
                nc.sync.dma_start(out=bnsc[L][:], in_=bnsc_in[L][:])
                bnsh[L] = cpool.tile([PF, 1], f32, tag=f"bnsh{L}", name=f"bnsh{L}")
                nc.sync.dma_start(out=bnsh[L][:], in_=bnsh_in[L][:])
            b3c = cpool.tile([PF, 1], f32, tag="b3c")
            nc.sync.dma_start(out=b3c[:], in_=b3_p[:])
            xr = cpool.tile([1, S], f32, tag="xr")
            nc.sync.dma_start(out=xr[:], in_=x_row[:])

            # giota [128,128]: every partition = 0..127 row
            io = cpool.tile([1, 128], f32, tag="io")
            nc.sync.dma_start(out=io[:], in_=iota_p[:])
            on = cpool.tile([1, 128], f32, tag="on")
            nc.sync.dma_start(out=on[:], in_=ones_p[:])
            gio_ps = ppool.tile([128, 128], f32, tag="onceps", bufs=1)
            nc.tensor.matmul(out=gio_ps[:], lhsT=on[:], rhs=io[:], start=True, stop=True)
            giota = cpool.tile([128, 128], f32, tag="giota")
            nc.vector.tensor_copy(out=giota[:], in_=gio_ps[:])

            # identity for PE transpose
            ident = cpool.tile([128, 128], f32, tag="ident")
            from concourse.masks import make_identity
            make_identity(nc, ident[:])

            # dinv/batch per tile: [128, T] host-precomputed images
            dinv = cpool.tile([128, T], f32, tag="dinv")
            nc.sync.dma_start(out=dinv[:], in_=deg_p[:])
            batc = cpool.tile([128, T], f32, tag="batc")
            nc.sync.dma_start(out=batc[:], in_=bat_p[:])

            # rcnt = 1/max(cnt,1)
            rcnt = cpool.tile([128, 1], f32, tag="rcnt")
            nc.sync.dma_start(out=rcnt[:], in_=cnt_p[:])
            nc.vector.tensor_scalar_max(out=rcnt[:], in0=rcnt[:], scalar1=1.0)
            nc.vector.reciprocal(out=rcnt[:], in_=rcnt[:])

            # ---- layer-1 table: T1 = dinv * (x @ W1), 4-tile batches ----
            for t0 in range(0, T, 4):
                ng = min(4, T - t0)
                ps_g = ppool.tile([128, 4, PF], f32, tag="tab_ps")
                for i in range(ng):
                    nc.tensor.matmul(
                        out=ps_g[:, i, :],
                        lhsT=xr[0:1, 128 * (t0 + i) : 128 * (t0 + i + 1)],
                        rhs=w1[:], start=True, stop=True,
                    )
                tt = wpool.tile([128, 4, PF], f32, tag="trow")
                nc.vector.tensor_tensor(
                    out=tt[:, 0:ng, :], in0=ps_g[:, 0:ng, :],
                    in1=dinv[:, t0 : t0 + ng].rearrange("p (g o) -> p g o", o=1)
                        .to_broadcast([128, ng, PF]),
                    op=mybir.AluOpType.mult)
                nc.sync.dma_start(
                    out=tloc[128 * t0 : 128 * (t0 + ng), :]
                        .rearrange("(g p) f -> p g f", g=ng),
                    in_=tt[:, 0:ng, :])

            if do_ag:
                nc.gpsimd.collective_compute(
                    "AllGather", mybir.AluOpType.bypass,
                    replica_groups=[list(range(N_CORES))],
                    ins=[tloc[:]], outs=[tabs[0][0:NT, :]],
                )

            # ---- layers ----
            qc = 0  # round-robin gather queue
            pool_ps = ppool1.tile([128, F], f32, tag="pool_ps")
            for L in list((1, 2, 3))[:n_layers]:
                for t in range(T):
                    Dt = int(D[t])
                    idx_t = wpool.tile([128, 8 * Dt], mybir.dt.int16, tag="idx")
                    nc.sync.dma_start(
                        out=idx_t[:],
                        in_=idx_p[:, 8 * tile_off[t] : 8 * (tile_off[t] + Dt)],
                    )
                    msg = mpool.tile([128, Dt, PF], f32, tag="msg", name="msg") if do_gather else None
                    for ci, (q, so, k) in enumerate(calls[t] if do_gather else []):
                        wo = woffs[t][ci] - 8 * tile_off[t]
                        nc.gpsimd.dma_gather(
                            out_ap=msg[:, so : so + k, :],
                            in_ap=tabs[(L - 1) % 2][CHUNK * q : CHUNK * (q + 1), :],
                            idxs_ap=idx_t[:, wo : wo + 8 * k],
                            num_idxs=128 * k,
                            num_idxs_reg=128 * k,
                            elem_size=PF,
                            queue_num=qc % N_QUEUES,
                        )
                        qc += 1
                    u = wpool.tile([128, F], f32, tag="u")
                    if not (do_reduce and do_gather):
                        nc.vector.memset(u[:], 0.0)
                    else:
                        nc.vector.tensor_reduce(
                        out=u[:],
                            in_=msg[:, :, 0:F].rearrange("p s f -> p f s"),
                            axis=mybir.AxisListType.X,
                            op=mybir.AluOpType.add,
                        )
                    v = wpool.tile([128, F], f32, tag="v")
                    nc.vector.tensor_tensor(
                        out=v[:], in0=u[:],
                        in1=dinv[:, t : t + 1].to_broadcast([128, F]),
                        op=mybir.AluOpType.mult,
                    )
                    if L < 3 and not do_tables:
                        pass
                    elif L < 3:
                        # transpose into group PSUM; BN+relu once per 4 tiles
                        g = t % 4
                        if g == 0:
                            vt_g = ppool.tile([F, 512], f32, tag="vt_g")
                        nc.tensor.transpose(out=vt_g[:, 128 * g : 128 * (g + 1)],
                                            in_=v[:], identity=ident[:])
                        if g == 3 or t == T - 1:
                            n_in_g = g + 1
                            W_ = 128 * n_in_g
                            ht = wpool.tile([F, 512], f32, tag="ht")
                            nc.vector.tensor_scalar(
                                out=ht[:, 0:W_], in0=vt_g[:, 0:W_],
                                scalar1=bnsc[L][0:F, :], scalar2=bnsh[L][0:F, :],
                                op0=mybir.AluOpType.mult, op1=mybir.AluOpType.add,
                            )
                            nc.scalar.activation(out=ht[:, 0:W_], in_=ht[:, 0:W_],
                                                 func=mybir.ActivationFunctionType.Relu)
                            wnext = w2 if L == 1 else w3
                            for i in range(n_in_g):
                                ti = t - n_in_g + 1 + i
                                ps = ppool.tile([128, PF], f32, tag="tab_ps")
                                nc.tensor.matmul(
                                    out=ps[:], lhsT=ht[:, 128 * i : 128 * (i + 1)],
                                    rhs=wnext[:], start=True, stop=True,
                                )
                                tt = wpool.tile([128, PF], f32, tag="trow")
                                nc.vector.tensor_tensor(
                                    out=tt[:], in0=ps[:],
                                    in1=dinv[:, ti : ti + 1].to_broadcast([128, PF]),
                                    op=mybir.AluOpType.mult,
                                )
                                nc.sync.dma_start(
                                    out=tloc[128 * ti : 128 * (ti + 1), :], in_=tt[:]
                                )
                    else:
                        # pool: P[n,g] = (bat[n]==g); pool_ps += P^T... lhsT=P
                        Pm = wpool.tile([128, 128], f32, tag="Pm")
                        nc.vector.tensor_tensor(
                            out=Pm[:], in0=batc[:, t : t + 1].to_broadcast([128, 128]),
                            in1=giota[:], op=mybir.AluOpType.is_equal,
                        )
                        nc.tensor.matmul(out=pool_ps[:], lhsT=Pm[:], rhs=v[:],
                                         start=(t == 0), stop=(t == T - 1))
                if L < 3 and do_ag:
                    nc.gpsimd.collective_compute(
                        "AllGather", mybir.AluOpType.bypass,
                        replica_groups=[list(range(N_CORES))],
                        ins=[tloc[:]], outs=[tabs[L % 2][0:NT, :]],
                    )

            # ---- finalize pool: /cnt, transpose, +b3 ----
            pm = wpool.tile([128, F], f32, tag="pm")
            nc.vector.tensor_tensor(out=pm[:], in0=pool_ps[:],
                                    in1=rcnt[:].to_broadcast([128, F]),
                                    op=mybir.AluOpType.mult)
            pt_ps = ppool.tile([F, 128], f32, tag="onceps", bufs=1)
            nc.tensor.transpose(out=pt_ps[:], in_=pm[:], identity=ident[:])
            ot = wpool.tile([F, 128], f32, tag="ot")
            nc.vector.tensor_scalar(out=ot[:], in0=pt_ps[:], scalar1=b3c[0:F, :],
                                    scalar2=None, op0=mybir.AluOpType.add)
            nc.sync.dma_start(out=out_p[:], in_=ot[:])

    nc.finalize()
    return nc


# --------------------------------------------------------------------------
# entry point
# --------------------------------------------------------------------------

def _make_in_maps(meta, inp):
    S = meta["S"]
    x = np.asarray(inp["x"], np.float32)

    def padF(a):  # [F] -> [PF,1]
        o = np.zeros((PF, 1), np.float32)
        o[:F, 0] = np.asarray(a, np.float32)
        return o

    def padW(w):  # [k,F] -> [k,PF]
        w = np.asarray(w, np.float32)
        o = np.zeros((w.shape[0], PF), np.float32)
        o[:, :F] = w
        return o

    # bn affine: h = relu(sc*(u'+bL) + sh) with u' = dinv*u; fold bL:
    def bn_fold(g, b_, m, v, bL):
        sc = np.asarray(g) / np.sqrt(np.asarray(v) + BN_EPS)
        sh = np.asarray(b_) - np.asarray(m) * sc + sc * np.asarray(bL)
        return padF(sc), padF(sh)

    bnsc1, bnsh1 = bn_fold(inp["bn1_g"], inp["bn1_b"], inp["bn1_m"], inp["bn1_v"], inp["b1"])
    bnsc2, bnsh2 = bn_fold(inp["bn2_g"], inp["bn2_b"], inp["bn2_m"], inp["bn2_v"], inp["b2"])

    iota = np.arange(128, dtype=np.float32)[None, :]
    ones = np.ones((1, 128), np.float32)

    in_maps = []
    st, loc_of = meta["st"], meta["loc_of"]
    for c in range(N_CORES):
        xl = np.zeros((1, S), np.float32)
        j = np.arange(st[c], st[c + 1])
        xl[0, loc_of[j]] = x[j, 0]
        degc = meta["deg_loc"][c][:, 0]
        dinv_im = np.where(degc > 0, 1.0 / np.sqrt(np.maximum(degc, 1.0)), 0.0)
        dinv_im = dinv_im.reshape(-1, 128).T.astype(np.float32).copy()
        bat_im = meta["bat_loc"][c][:, 0].reshape(-1, 128).T.astype(np.float32).copy()
        in_maps.append({
            "x_row": xl,
            "deg": dinv_im,
            "bat": bat_im,
            "cnt": meta["cnt_loc"][c],
            "idximg": meta["wimgs"][c],
            "W1p": padW(inp["W1"]), "W2p": padW(inp["W2"]), "W3p": padW(inp["W3"]),
            "bnsc1": bnsc1, "bnsh1": bnsh1,
            "bnsc2": bnsc2, "bnsh2": bnsh2,
            "b3": padF(inp["b3"]),
            "iota": iota, "ones": ones,
        })
    return in_maps


def kernel(x, edge_index, batch, W1, b1, bn1_g, bn1_b, bn1_m, bn1_v,
           W2, b2, bn2_g, bn2_b, bn2_m, bn2_v, W3, b3):
    from concourse.bass_utils import run_bass_kernel_spmd

    edge_index = np.asarray(edge_index)
    batch_np = np.asarray(batch)

    key = (edge_index.shape[1], int(edge_index[0, :8].sum()), int(batch_np[:8].sum()))
    if key not in _CACHE:
        meta = _prep(edge_index, batch_np)
        nc = _build(meta)
        _CACHE[key] = (meta, nc)
    meta, nc = _CACHE[key]

    inp = dict(x=x, W1=W1, b1=b1, bn1_g=bn1_g, bn1_b=bn1_b, bn1_m=bn1_m,
               bn1_v=bn1_v, W2=W2, b2=b2, bn2_g=bn2_g, bn2_b=bn2_b,
               bn2_m=bn2_m, bn2_v=bn2_v, W3=W3, b3=b3)
    in_maps = _make_in_maps(meta, inp)

    res = run_bass_kernel_spmd(nc, in_maps, list(range(N_CORES)))
    out = np.empty((N_GRAPHS, F), np.float32)
    for c in range(N_CORES):
        out[GPC * c : GPC * (c + 1), :] = res.results[c]["out"].T
    return out



# revision 4
# speedup vs baseline: 1.0084x; 1.0084x over previous
"""GCN (3-layer) + global mean pool on 8 Trainium2 NeuronCores.

Sharding: 1024 graphs -> 8 shards of 128 graphs (contiguous node ranges,
batch is sorted). Each core owns its shard's nodes and all edges whose dst
lands in the shard. Per layer, each core computes the per-node linear table
T = dinv * (h @ W) for its own nodes, an AllGather replicates the full table,
then dma_gather pulls T[src] rows per edge (ELL slots per 128-node tile,
split by int16 chunk of the global table) and a strided DVE reduce sums each
node's slots. norm factorizes as dinv[src]*dinv[dst]: dinv[src] is folded
into the table, dinv[dst] is applied after the reduce.
"""

import numpy as np

N_NODES = 100000
N_GRAPHS = 1024
N_CORES = 8
GPC = N_GRAPHS // N_CORES  # graphs per core
F = 32                     # true feature width
PF = 64                    # padded row width (256B dma_gather elem)
CHUNK = 32768              # int16-addressable table rows per gather chunk
BN_EPS = 1e-5
import os
MAX_SLOTS_PER_CALL = int(os.environ.get("K_SLOTS", "8"))  # 1024-idx dma_gather runtime cap / 128
N_QUEUES = int(os.environ.get("K_QUEUES", "4"))
SCRATCH = int(os.environ.get("K_SCRATCH", "81920"))

_CACHE = {}


# --------------------------------------------------------------------------
# host-side prep: sharding, ELL layout, wrapped index image
# --------------------------------------------------------------------------

def _prep(edge_index, batch):
    src_g = edge_index[0].astype(np.int64)
    dst_g = edge_index[1].astype(np.int64)
    batch = batch.astype(np.int64)
    n = N_NODES

    # in-degree including self loop (= reference deg)
    deg = np.bincount(dst_g, minlength=n).astype(np.int64) + 1

    # node ranges per core: graphs [128c, 128c+128)
    gcounts = np.bincount(batch, minlength=N_GRAPHS)
    gends = np.cumsum(gcounts)
    st = np.zeros(N_CORES + 1, np.int64)
    for c in range(1, N_CORES + 1):
        st[c] = gends[GPC * c - 1]
    sizes = np.diff(st)
    S = (int(sizes.max()) // 128 + 1) * 128  # >=1 pad row per shard
    T = S // 128

    # per-core degree sort (desc) -> local position of each node
    loc_of = np.empty(n, np.int64)
    for c in range(N_CORES):
        j = np.arange(st[c], st[c + 1])
        order = np.argsort(-deg[j], kind="stable")
        loc_of[j[order]] = np.arange(sizes[c])
    owner = np.searchsorted(st[1:], np.arange(n), side="right")
    owner_of = lambda v: np.searchsorted(st[1:], v, side="right")
    row_of = S * owner + loc_of  # global table row of node

    # all edges incl self loops, routed to dst owner
    all_src = np.concatenate([src_g, np.arange(n)])
    all_dst = np.concatenate([dst_g, np.arange(n)])
    e_owner = owner[all_dst]
    e_srcrow = row_of[all_src]
    e_dstloc = loc_of[all_dst]

    # pad (zero) rows per chunk: any never-written-by-a-real-node row < 8S
    used = np.zeros(N_CORES * S, bool)
    used[row_of] = True
    pad_rows = np.nonzero(~used)[0]
    Zq = np.empty(4, np.int64)
    for q in range(4):
        cand = pad_rows[(pad_rows >= CHUNK * q) & (pad_rows < CHUNK * (q + 1))]
        assert len(cand) > 0, f"no pad row in chunk {q}"
        Zq[q] = cand[0]

    # per-core (tile, chunk) slot counts -> common maxima
    counts = np.zeros((N_CORES, T * 128, 4), np.int32)
    core_edges = []
    for c in range(N_CORES):
        m = e_owner == c
        sr, dl = e_srcrow[m], e_dstloc[m]
        q = sr // CHUNK
        np.add.at(counts[c], (dl, q), 1)
        core_edges.append((sr, dl, q))
    cnt_tiles = counts.reshape(N_CORES, T, 128, 4)
    stq = cnt_tiles.max(axis=(0, 2)).astype(np.int64)  # [T, 4] common
    D = stq.sum(axis=1)                                # [T] slots per tile
    qoff = np.cumsum(stq, axis=1) - stq                # [T, 4] slot offset of chunk q
    tile_off = np.concatenate([[0], np.cumsum(D)])     # [T+1]
    TOT = int(tile_off[-1])

    # gather call structure (common): per tile, list of (q, slot_off, k_slots)
    calls = []
    for t in range(T):
        cl = []
        for q in range(4):
            s = int(stq[t, q])
            so = int(qoff[t, q])
            while s > 0:
                k = min(MAX_SLOTS_PER_CALL, s)
                cl.append((q, so, k))
                so += k
                s -= k
        calls.append(cl)

    # per-core flat ELL image [128, TOT] int16 (slot-major per tile)
    imgs = []
    for c in range(N_CORES):
        sr, dl, q = core_edges[c]
        img = np.empty((128, TOT), np.int16)
        for t in range(T):
            for qq in range(4):
                img[:, tile_off[t] + qoff[t, qq] : tile_off[t] + qoff[t, qq] + stq[t, qq]] = (
                    Zq[qq] - CHUNK * qq
                )
        # rank of edge within its (dst, chunk) group
        key = dl * 4 + q
        order = np.argsort(key, kind="stable")
        sk = key[order]
        starts = np.concatenate([[0], np.nonzero(np.diff(sk))[0] + 1])
        grp_start = np.zeros(len(sk), np.int64)
        grp_start[starts] = np.arange(len(sk))[starts]
        grp_start = np.maximum.accumulate(grp_start)
        rank = np.arange(len(sk)) - grp_start
        sro, dlo, qo = sr[order], dl[order], q[order]
        t_ = dlo // 128
        p_ = dlo % 128
        slot = qoff[t_, qo] + rank
        img[p_, tile_off[t_] + slot] = (sro - CHUNK * qo).astype(np.int16)
        imgs.append(img)

    # wrap for dma_gather idx layout: per call, [128, 8k] block where
    # [pp, i] = logical[(i*16 + pp%16)]  with logical j -> (slot j//128, part j%128)
    wrap_cols = 8 * TOT
    wimgs = [np.empty((128, wrap_cols), np.int16) for _ in range(N_CORES)]
    pp = np.arange(128)[:, None]
    woff_of_call = []  # per tile: list of wrapped col offsets aligned with calls
    for t in range(T):
        woffs = []
        for (q, so, k) in calls[t]:
            woffs.append(8 * (tile_off[t] + so))
            i = np.arange(8 * k)[None, :]
            j = i * 16 + (pp % 16)
            for c in range(N_CORES):
                blk = imgs[c][:, tile_off[t] + so : tile_off[t] + so + k]
                wimgs[c][:, 8 * (tile_off[t] + so) : 8 * (tile_off[t] + so + k)] = blk[
                    j % 128, j // 128
                ]
        woff_of_call.append(woffs)

    # per-core aux arrays (local order, padded to S)
    deg_loc = np.zeros((N_CORES, S, 1), np.float32)
    bat_loc = np.full((N_CORES, S, 1), 1000.0, np.float32)
    cnt_loc = np.zeros((N_CORES, 128, 1), np.float32)
    for c in range(N_CORES):
        j = np.arange(st[c], st[c + 1])
        deg_loc[c, loc_of[j], 0] = deg[j]
        bat_loc[c, loc_of[j], 0] = batch[j] - GPC * c
        cnt_loc[c, :, 0] = gcounts[GPC * c : GPC * (c + 1)]

    return dict(
        S=S, T=T, stq=stq, D=D, tile_off=tile_off, TOT=TOT, calls=calls,
        woff_of_call=woff_of_call, wimgs=wimgs, imgs=imgs, qoff=qoff,
        deg_loc=deg_loc, bat_loc=bat_loc, cnt_loc=cnt_loc, loc_of=loc_of,
        st=st, Zq=Zq, row_of=row_of,
    )


# --------------------------------------------------------------------------
# walrus workaround: at most one sem-wait per instruction
# --------------------------------------------------------------------------

def _install_tile_patch():
    import concourse.mybir as mybir
    from concourse.tile import TileContext
    from concourse.vector_clock import ScopedClock

    if getattr(TileContext, "_wait_split_installed", False):
        return

    def split_all_waits(nc):
        for bb in nc.main_func.blocks:
            insts = list(bb.instructions)
            if not any(
                i.sync_info is not None and len(i.sync_info.on_wait) > 1
                for i in insts
            ):
                continue
            newlist = []
            tail_bb = nc.cur_bb.bb if nc.cur_bb is not None else None
            for inst in insts:
                w = list(inst.sync_info.on_wait) if inst.sync_info is not None else []
                if len(w) > 1 and inst.engine != mybir.EngineType.Unassigned:
                    extra, keep = w[:-1], w[-1:]
                    inst.sync_info.on_wait = keep
                    eng = nc.engines[inst.engine]
                    for wi in extra:
                        nop = eng.nop(nofuse=True, hint="wait_split")
                        ni = nop.ins if hasattr(nop, "ins") else nop
                        if tail_bb is not None and ni in tail_bb.instructions:
                            tail_bb.instructions.remove(ni)
                        if ni.sync_info is None:
                            ni.sync_info = mybir.SyncInfo(on_wait=[], on_update=[])
                        ni.sync_info.on_wait = [wi]
                        ni.sync_info.on_update = []
                        newlist.append(ni)
                newlist.append(inst)
            bb.instructions.clear()
            for x in newlist:
                bb.instructions.append(x)

    def _patched(self, tick_clock, wait_clock):
        drain_inst = self.nc.sync.drain()
        wait_clock.add_sem_waits(
            drain_inst.ins, ScopedClock({None: tick_clock.global_clock})
        )
        self.nc.all_engine_barrier()
        assert self.sems is not None
        popped = self.nc._tile_sem_poison_stack.pop()
        assert popped is self._sem_poison
        self.nc.clear_and_free_semaphores(list(self.sems.allocated().values()))
        self.nc.all_engine_barrier()
        _requeue_gather_sems(self.nc)
        split_all_waits(self.nc)

    TileContext._drain_and_barrier = _patched
    TileContext._wait_split_installed = True


def _requeue_gather_sems(nc):
    """Make multi-queue dma_gather sound.

    The Tile scheduler counts all SWDGE completions on rotating DMASW sems and
    waits assume FIFO completion order - false across queues. Rewrite: each
    gather updates a per-queue cumulative sem; every consumer wait on a DMASW
    sem (value 16m = "m-th update of that sem") is mapped to its program-order
    dependency prefix N and replaced by per-queue waits (count of prefix on
    each queue). Producer-side sem-slot-reuse waits are dropped (cumulative
    sems are never reset mid-program).
    """
    import copy
    if N_QUEUES <= 1 or not os.environ.get("K_SURGERY"):
        return
    gathers = []
    for bb in nc.main_func.blocks:
        for inst in bb.instructions:
            if type(inst).__name__ == "InstDMAGatherAnt":
                gathers.append(inst)
    if not gathers:
        return
    swsem_ids = set()
    for g in gathers:
        for u in g.sync_info.on_update:
            if str(getattr(u, "ant_name", "")).startswith("DMASW"):
                swsem_ids.add(u.id)
    # no non-gather instruction may update these sems mid-program
    for bb in nc.main_func.blocks:
        for inst in bb.instructions:
            if type(inst).__name__ == "InstDMAGatherAnt" or inst.sync_info is None:
                continue
            for u in inst.sync_info.on_update:
                assert getattr(u, "id", None) not in swsem_ids, (
                    f"non-gather updates DMASW sem: {inst}"
                )
    upd_order = {s: [] for s in swsem_ids}
    for i, g in enumerate(gathers):
        for u in g.sync_info.on_update:
            if u.id in swsem_ids:
                upd_order[u.id].append(i)
    qsems = sorted(swsem_ids)[:N_QUEUES]
    q_of = [int(g.queue_num) for g in gathers]
    prefix = [(0,) * N_QUEUES]
    cur = [0] * N_QUEUES
    for q in q_of:
        cur[q] += 1
        prefix.append(tuple(cur))

    proto_wait = None
    for bb in nc.main_func.blocks:
        for inst in bb.instructions:
            if inst.sync_info is None:
                continue
            for w in inst.sync_info.on_wait:
                if getattr(w, "id", None) in swsem_ids:
                    proto_wait = w
                    break
            if proto_wait is not None:
                break
        if proto_wait is not None:
            break

    # rewrite gather updates to per-queue cumulative sems; drop their DMASW waits
    # (replace with fresh copies - sync objects may be shared across insts)
    for i, g in enumerate(gathers):
        new_upd = []
        for u in g.sync_info.on_update:
            if u.id in swsem_ids:
                nu = copy.deepcopy(u)
                nu.id = qsems[q_of[i]]
                nu.ant_name = f"QSW{q_of[i]}"
                new_upd.append(nu)
            else:
                new_upd.append(u)
        g.sync_info.on_update = new_upd
        g.sync_info.on_wait = [
            w for w in g.sync_info.on_wait if getattr(w, "id", None) not in swsem_ids
        ]

    # rewrite consumer waits
    for bb in nc.main_func.blocks:
        for inst in bb.instructions:
            if type(inst).__name__ == "InstDMAGatherAnt" or inst.sync_info is None:
                continue
            waits = list(inst.sync_info.on_wait)
            sw = [w for w in waits if getattr(w, "id", None) in swsem_ids]
            if not sw:
                continue
            N = 0
            for w in sw:
                m = int(w.wait_value) // 16
                order = upd_order[w.id]
                if m <= 0 or not order:
                    continue
                m = min(m, len(order))
                N = max(N, order[m - 1] + 1)
            keep = [w for w in waits if getattr(w, "id", None) not in swsem_ids]
            for q in range(N_QUEUES):
                c = prefix[N][q]
                if c > 0:
                    nw = copy.deepcopy(proto_wait)
                    nw.id = qsems[q]
                    nw.ant_name = f"QSW{q}"
                    nw.wait_value = 16 * c
                    keep.append(nw)
            inst.sync_info.on_wait = keep


# --------------------------------------------------------------------------
# device program
# --------------------------------------------------------------------------

def _build(meta, n_layers=3, do_gather=True, do_reduce=True, do_tables=True, do_ag=True):
    import concourse.bacc as bacc
    import concourse.mybir as mybir
    from concourse.tile import TileContext

    _install_tile_patch()

    S, T = meta["S"], meta["T"]
    calls, woffs, tile_off, D = (
        meta["calls"], meta["woff_of_call"], meta["tile_off"], meta["D"],
    )
    TOT = meta["TOT"]
    NT = N_CORES * S  # real table rows
    f32 = mybir.dt.float32

    nc = bacc.Bacc(None, target_bir_lowering=False, num_swdge_queues=N_QUEUES,
                   dynamic_dma_scratch_size=SCRATCH)
    P_ = nc.declare_dram_parameter

    x_row = P_("x_row", [1, S], f32, isOutput=False)
    deg_p = P_("deg", [128, S // 128], f32, isOutput=False)
    bat_p = P_("bat", [128, S // 128], f32, isOutput=False)
    cnt_p = P_("cnt", [128, 1], f32, isOutput=False)
    idx_p = P_("idximg", [128, 8 * TOT], mybir.dt.int16, isOutput=False)
    w1_p = P_("W1p", [1, PF], f32, isOutput=False)
    w2_p = P_("W2p", [F, PF], f32, isOutput=False)
    w3_p = P_("W3p", [F, PF], f32, isOutput=False)
    bnsc_in = [None, P_("bnsc1", [PF, 1], f32, isOutput=False),
               P_("bnsc2", [PF, 1], f32, isOutput=False)]
    bnsh_in = [None, P_("bnsh1", [PF, 1], f32, isOutput=False),
               P_("bnsh2", [PF, 1], f32, isOutput=False)]
    b3_p = P_("b3", [PF, 1], f32, isOutput=False)
    iota_p = P_("iota", [1, 128], f32, isOutput=False)
    ones_p = P_("ones", [1, 128], f32, isOutput=False)
    out_p = P_("out", [F, 128], f32, isOutput=True)

    tloc = nc.dram_tensor("tloc", [S, PF], f32)
    # double-buffered gather table: AG for layer L+1 must not overwrite the
    # buffer still being read by layer L's (queue-reordered) gathers
    tabs = [
        nc.dram_tensor("tabA", [4 * CHUNK, PF], f32, addr_space="Shared"),
        nc.dram_tensor("tabB", [4 * CHUNK, PF], f32, addr_space="Shared"),
    ]

    with TileContext(nc) as tc:
        with (
            tc.tile_pool(name="const", bufs=1) as cpool,
            tc.tile_pool(name="work", bufs=3) as wpool,
            tc.tile_pool(name="msg", bufs=3) as mpool,
            tc.tile_pool(name="psum", bufs=2, space="PSUM") as ppool,
            tc.tile_pool(name="psum1", bufs=1, space="PSUM") as ppool1,
        ):
            # ---- constants ----
            w1 = cpool.tile([1, PF], f32, tag="w1")
            nc.sync.dma_start(out=w1[:], in_=w1_p[:])
            w2 = cpool.tile([F, PF], f32, tag="w2")
            nc.sync.dma_start(out=w2[:], in_=w2_p[:])
            w3 = cpool.tile([F, PF], f32, tag="w3")
            nc.sync.dma_start(out=w3[:], in_=w3_p[:])
            bnsc = [None, None, None]
            bnsh = [None, None, None]
            for L in (1, 2):
                bnsc[L] = cpool.tile([PF, 1], f32, tag=f"bnsc{L}", name=f"bnsc{L}")
                nc.sync.dma_start(out=bnsc[L][:], in_=bnsc_in[L][:])
                bnsh[L] = cpool.tile([PF, 1], f32, tag=f"bnsh{L}", name=f"bnsh{L}")
                nc.sync.dma_start(out=bnsh[L][:], in_=bnsh_in[L][:])
            b3c = cpool.tile([PF, 1], f32, tag="b3c")
            nc.sync.dma_start(out=b3c[:], in_=b3_p[:])
            xr = cpool.tile([1, S], f32, tag="xr")
            nc.sync.dma_start(out=xr[:], in_=x_row[:])

            # giota [128,128]: every partition = 0..127 row
            io = cpool.tile([1, 128], f32, tag="io")
            nc.sync.dma_start(out=io[:], in_=iota_p[:])
            on = cpool.tile([1, 128], f32, tag="on")
            nc.sync.dma_start(out=on[:], in_=ones_p[:])
            gio_ps = ppool.tile([128, 128], f32, tag="onceps", bufs=1)
            nc.tensor.matmul(out=gio_ps[:], lhsT=on[:], rhs=io[:], start=True, stop=True)
            giota = cpool.tile([128, 128], f32, tag="giota")
            nc.vector.tensor_copy(out=giota[:], in_=gio_ps[:])

            # identity for PE transpose
            ident = cpool.tile([128, 128], f32, tag="ident")
            from concourse.masks import make_identity
            make_identity(nc, ident[:])

            # dinv/batch per tile: [128, T] host-precomputed images
            dinv = cpool.tile([128, T], f32, tag="dinv")
            nc.sync.dma_start(out=dinv[:], in_=deg_p[:])
            batc = cpool.tile([128, T], f32, tag="batc")
            nc.sync.dma_start(out=batc[:], in_=bat_p[:])

            # rcnt = 1/max(cnt,1)
            rcnt = cpool.tile([128, 1], f32, tag="rcnt")
            nc.sync.dma_start(out=rcnt[:], in_=cnt_p[:])
            nc.vector.tensor_scalar_max(out=rcnt[:], in0=rcnt[:], scalar1=1.0)
            nc.vector.reciprocal(out=rcnt[:], in_=rcnt[:])

            # ---- layer-1 table: T1 = dinv * (x @ W1), 4-tile batches ----
            for t0 in range(0, T, 4):
                ng = min(4, T - t0)
                ps_g = ppool.tile([128, 4, PF], f32, tag="tab_ps")
                for i in range(ng):
                    nc.tensor.matmul(
                        out=ps_g[:, i, :],
                        lhsT=xr[0:1, 128 * (t0 + i) : 128 * (t0 + i + 1)],
                        rhs=w1[:], start=True, stop=True,
                    )
                tt = wpool.tile([128, 4, PF], f32, tag="trow")
                nc.vector.tensor_tensor(
                    out=tt[:, 0:ng, :], in0=ps_g[:, 0:ng, :],
                    in1=dinv[:, t0 : t0 + ng].rearrange("p (g o) -> p g o", o=1)
                        .to_broadcast([128, ng, PF]),
                    op=mybir.AluOpType.mult)
                nc.sync.dma_start(
                    out=tloc[128 * t0 : 128 * (t0 + ng), :]
                        .rearrange("(g p) f -> p g f", g=ng),
                    in_=tt[:, 0:ng, :])

            if do_ag:
                nc.gpsimd.collective_compute(
                    "AllGather", mybir.AluOpType.bypass,
                    replica_groups=[list(range(N_CORES))],
                    ins=[tloc[:]], outs=[tabs[0][0:NT, :]],
                )

            # ---- layers ----
            qc = 0  # round-robin gather queue
            pool_ps = ppool1.tile([128, F], f32, tag="pool_ps")
            for L in list((1, 2, 3))[:n_layers]:
                for t in range(T):
                    Dt = int(D[t])
                    idx_t = wpool.tile([128, 8 * Dt], mybir.dt.int16, tag="idx")
                    nc.sync.dma_start(
                        out=idx_t[:],
                        in_=idx_p[:, 8 * tile_off[t] : 8 * (tile_off[t] + Dt)],
                    )
                    msg = mpool.tile([128, Dt, PF], f32, tag="msg", name="msg") if do_gather else None
                    for ci, (q, so, k) in enumerate(calls[t] if do_gather else []):
                        wo = woffs[t][ci] - 8 * tile_off[t]
                        nc.gpsimd.dma_gather(
                            out_ap=msg[:, so : so + k, :],
                            in_ap=tabs[(L - 1) % 2][CHUNK * q : CHUNK * (q + 1), :],
                            idxs_ap=idx_t[:, wo : wo + 8 * k],
                            num_idxs=128 * k,
                            num_idxs_reg=128 * k,
                            elem_size=PF,
                            queue_num=qc % N_QUEUES,
                        )
                        qc += 1
                    u = wpool.tile([128, F], f32, tag="u")
                    if not (do_reduce and do_gather):
                        nc.vector.memset(u[:], 0.0)
                    else:
                        nc.vector.tensor_reduce(
                        out=u[:],
                            in_=msg[:, :, 0:F].rearrange("p s f -> p f s"),
                            axis=mybir.AxisListType.X,
                            op=mybir.AluOpType.add,
                        )
                    v = wpool.tile([128, F], f32, tag="v")
                    nc.vector.tensor_tensor(
                        out=v[:], in0=u[:],
                        in1=dinv[:, t : t + 1].to_broadcast([128, F]),
                        op=mybir.AluOpType.mult,
                    )
                    if L < 3 and not do_tables:
                        pass
                    elif L < 3:
                        # transpose into group PSUM; BN+relu once per 4 tiles
                        g = t % 4
                        if g == 0:
                            vt_g = ppool.tile([F, 512], f32, tag="vt_g")
                        nc.tensor.transpose(out=vt_g[:, 128 * g : 128 * (g + 1)],
                                            in_=v[:], identity=ident[:])
                        if g == 3 or t == T - 1:
                            n_in_g = g + 1
                            W_ = 128 * n_in_g
                            ht = wpool.tile([F, 512], f32, tag="ht")
                            nc.vector.tensor_scalar(
                                out=ht[:, 0:W_], in0=vt_g[:, 0:W_],
                                scalar1=bnsc[L][0:F, :], scalar2=bnsh[L][0:F, :],
                                op0=mybir.AluOpType.mult, op1=mybir.AluOpType.add,
                            )
                            nc.scalar.activation(out=ht[:, 0:W_], in_=ht[:, 0:W_],
                                                 func=mybir.ActivationFunctionType.Relu)
                            wnext = w2 if L == 1 else w3
                            for i in range(n_in_g):
                                ti = t - n_in_g + 1 + i
                                ps = ppool.tile([128, PF], f32, tag="tab_ps")
                                nc.tensor.matmul(
                                    out=ps[:], lhsT=ht[:, 128 * i : 128 * (i + 1)],
                                    rhs=wnext[:], start=True, stop=True,
                                )
                                tt = wpool.tile([128, PF], f32, tag="trow")
                                nc.vector.tensor_tensor(
                                    out=tt[:], in0=ps[:],
                                    in1=dinv[:, ti : ti + 1].to_broadcast([128, PF]),
                                    op=mybir.AluOpType.mult,
                                )
                                nc.sync.dma_start(
                                    out=tloc[128 * ti : 128 * (ti + 1), :], in_=tt[:]
                                )
                    else:
                        # pool: P[n,g] = (bat[n]==g); pool_ps += P^T... lhsT=P
                        Pm = wpool.tile([128, 128], f32, tag="Pm")
                        nc.vector.tensor_tensor(
                            out=Pm[:], in0=batc[:, t : t + 1].to_broadcast([128, 128]),
                            in1=giota[:], op=mybir.AluOpType.is_equal,
                        )
                        nc.tensor.matmul(out=pool_ps[:], lhsT=Pm[:], rhs=v[:],
                                         start=(t == 0), stop=(t == T - 1))
                if L < 3 and do_ag:
                    nc.gpsimd.collective_compute(
                        "AllGather", mybir.AluOpType.bypass,
                        replica_groups=[list(range(N_CORES))],
                        ins=[tloc[:]], outs=[tabs[L % 2][0:NT, :]],
                    )

            # ---- finalize pool: /cnt, transpose, +b3 ----
            pm = wpool.tile([128, F], f32, tag="pm")
            nc.vector.tensor_tensor(out=pm[:], in0=pool_ps[:],
                                    in1=rcnt[:].to_broadcast([128, F]),
                                    op=mybir.AluOpType.mult)
            pt_ps = ppool.tile([F, 128], f32, tag="onceps", bufs=1)
            nc.tensor.transpose(out=pt_ps[:], in_=pm[:], identity=ident[:])
            ot = wpool.tile([F, 128], f32, tag="ot")
            nc.vector.tensor_scalar(out=ot[:], in0=pt_ps[:], scalar1=b3c[0:F, :],
                                    scalar2=None, op0=mybir.AluOpType.add)
            nc.sync.dma_start(out=out_p[:], in_=ot[:])

    nc.finalize()
    return nc


# --------------------------------------------------------------------------
# entry point
# --------------------------------------------------------------------------

def _make_in_maps(meta, inp):
    S = meta["S"]
    x = np.asarray(inp["x"], np.float32)

    def padF(a):  # [F] -> [PF,1]
        o = np.zeros((PF, 1), np.float32)
        o[:F, 0] = np.asarray(a, np.float32)
        return o

    def padW(w):  # [k,F] -> [k,PF]
        w = np.asarray(w, np.float32)
        o = np.zeros((w.shape[0], PF), np.float32)
        o[:, :F] = w
        return o

    # bn affine: h = relu(sc*(u'+bL) + sh) with u' = dinv*u; fold bL:
    def bn_fold(g, b_, m, v, bL):
        sc = np.asarray(g) / np.sqrt(np.asarray(v) + BN_EPS)
        sh = np.asarray(b_) - np.asarray(m) * sc + sc * np.asarray(bL)
        return padF(sc), padF(sh)

    bnsc1, bnsh1 = bn_fold(inp["bn1_g"], inp["bn1_b"], inp["bn1_m"], inp["bn1_v"], inp["b1"])
    bnsc2, bnsh2 = bn_fold(inp["bn2_g"], inp["bn2_b"], inp["bn2_m"], inp["bn2_v"], inp["b2"])

    iota = np.arange(128, dtype=np.float32)[None, :]
    ones = np.ones((1, 128), np.float32)

    in_maps = []
    st, loc_of = meta["st"], meta["loc_of"]
    for c in range(N_CORES):
        xl = np.zeros((1, S), np.float32)
        j = np.arange(st[c], st[c + 1])
        xl[0, loc_of[j]] = x[j, 0]
        degc = meta["deg_loc"][c][:, 0]
        dinv_im = np.where(degc > 0, 1.0 / np.sqrt(np.maximum(degc, 1.0)), 0.0)
        dinv_im = dinv_im.reshape(-1, 128).T.astype(np.float32).copy()
        bat_im = meta["bat_loc"][c][:, 0].reshape(-1, 128).T.astype(np.float32).copy()
        in_maps.append({
            "x_row": xl,
            "deg": dinv_im,
            "bat": bat_im,
            "cnt": meta["cnt_loc"][c],
            "idximg": meta["wimgs"][c],
            "W1p": padW(inp["W1"]), "W2p": padW(inp["W2"]), "W3p": padW(inp["W3"]),
            "bnsc1": bnsc1, "bnsh1": bnsh1,
            "bnsc2": bnsc2, "bnsh2": bnsh2,
            "b3": padF(inp["b3"]),
            "iota": iota, "ones": ones,
        })
    return in_maps


def kernel(x, edge_index, batch, W1, b1, bn1_g, bn1_b, bn1_m, bn1_v,
           W2, b2, bn2_g, bn2_b, bn2_m, bn2_v, W3, b3):
    from concourse.bass_utils import run_bass_kernel_spmd

    edge_index = np.asarray(edge_index)
    batch_np = np.asarray(batch)

    key = (edge_index.shape[1], int(edge_index[0, :8].sum()), int(batch_np[:8].sum()))
    if key not in _CACHE:
        meta = _prep(edge_index, batch_np)
        nc = _build(meta)
        _CACHE[key] = (meta, nc)
    meta, nc = _CACHE[key]

    inp = dict(x=x, W1=W1, b1=b1, bn1_g=bn1_g, bn1_b=bn1_b, bn1_m=bn1_m,
               bn1_v=bn1_v, W2=W2, b2=b2, bn2_g=bn2_g, bn2_b=bn2_b,
               bn2_m=bn2_m, bn2_v=bn2_v, W3=W3, b3=b3)
    in_maps = _make_in_maps(meta, inp)

    res = run_bass_kernel_spmd(nc, in_maps, list(range(N_CORES)))
    out = np.empty((N_GRAPHS, F), np.float32)
    for c in range(N_CORES):
        out[GPC * c : GPC * (c + 1), :] = res.results[c]["out"].T
    return out

